# revision 1
# baseline (speedup 1.0000x reference)
"""RWKV time-mixing (C=4096) on 8 trn2 NeuronCores.

Strategy (tensor-parallel over channels, M=8 cores, S=C/M=512):
  - Core c owns channels sl = [c*512, (c+1)*512).
  - Phase 1: kk/vv/rr shards: kw[sl,:] @ xk etc. Weights are host-transposed
    so the contraction dim lands on SBUF partitions; the x-vector column is
    the (tiny) stationary operand, weight tiles stream as the moving operand
    at N=512 in float32r mode (1 cycle/row on the PE).
  - WKV recurrence: purely elementwise on the 512-channel shard, done in a
    [128, 4] layout (channel = j*128 + p).
  - Phase 2: partial out = ow[:, sl] @ (r*wkv): 4 k-tiles x 8 n-banks of
    matmuls into a [1, 4096] PSUM row; host sums the 8 per-core partials
    (the "all-reduce" of the column-sharded matvec).
  - new_state = x exactly (token shift), assembled on host.

k-index convention (phase 1): k = p*32 + n  (p = partition, n = k-tile id),
so W.T.reshape(128, 32, 512) puts k-tile n at [:, n, :] with contraction on
partitions, and x.reshape(128, 32) column n is the matching stationary vec.

The [1,512] -> [128,4] reshape of the phase-1 results runs on the PE: a
matmul whose stationary operand is a zero-padded [128,128] slice with the
data in partition 0 and whose moving operand is the unit vector e0 emits
the row slice as a [128,2] PSUM column pair (channel = j*128 + p). Phase 2
contracts k-tile tt=j over rows [j*128,(j+1)*128) of ow[:, sl].T, matching
that layout.
"""

import numpy as np

import concourse.bass as bass
import concourse.mybir as mybir
import concourse.tile as tile
from concourse import bacc, bass_utils

C = 4096
NCORES = 8
S = C // NCORES          # 512 channels per core
P = 128
KT = C // P              # 32 k-tiles in phase 1
KSUB = 4                 # k-tiles per DMA chunk (1 MB chunks)
NCHUNK = KT // KSUB      # 8 chunks per phase-1 matrix
OW_KT = S // P           # 4 k-tiles in phase 2
OW_HALF = C // 2         # ow chunks split into 1 MB column halves

F32 = mybir.dt.float32
F32R = mybir.dt.float32r
AF = mybir.ActivationFunctionType
MM_DTYPE = F32R          # float32r: 1 cycle/row at N>=512 (fp32 would be 4)

# xvecs layout: [128, 98] = xk[0:32] | xv[32:64] | xr[64:96] | e0[96:98] (f32r)
# (e0 = unit vector [1,0,...,0] + a zero column: N=2 moving operand of the
# PE-reshape matmuls — fp32r matmuls require an even moving free size)
# svecs layout: [128, 20] = aa[0:4] | bb[4:8] | pp[8:12] | tf[12:16] | td[16:20]
XVECS_W = 3 * KT + 2
SVECS_W = 5 * 4
OFF_XK, OFF_XV, OFF_XR, OFF_E0 = 0, KT, 2 * KT, 3 * KT
OFF_AA, OFF_BB, OFF_PP, OFF_TF, OFF_TD = 0, 4, 8, 12, 16


def _build():
    nc = bacc.Bacc("TRN2", target_bir_lowering=False, debug=False,
                   num_devices=NCORES)

    # weight/x tensors are declared float32r end-to-end: same 32-bit layout
    # (numpy side stays float32), but the BIR verifier requires fp32r matmul
    # operands to be produced as fp32r.
    xvecs_d = nc.dram_tensor("xvecs", [P, XVECS_W], MM_DTYPE, kind="ExternalInput")
    svecs_d = nc.dram_tensor("svecs", [P, SVECS_W], F32, kind="ExternalInput")
    wr_d = nc.dram_tensor("wr", [P, KT, S], MM_DTYPE, kind="ExternalInput")
    wk_d = nc.dram_tensor("wk", [P, KT, S], MM_DTYPE, kind="ExternalInput")
    wv_d = nc.dram_tensor("wv", [P, KT, S], MM_DTYPE, kind="ExternalInput")
    wo_d = nc.dram_tensor("wo", [OW_KT, P, C], MM_DTYPE, kind="ExternalInput")

    partial_d = nc.dram_tensor("partial", [1, C], F32, kind="ExternalOutput")
    nst_d = nc.dram_tensor("nst", [P, 12], F32, kind="ExternalOutput")

    with tile.TileContext(nc) as tc:
        with (
            tc.tile_pool(name="const", bufs=1) as const,
            tc.tile_pool(name="w", bufs=13) as wpool,
            tc.tile_pool(name="ow", bufs=8) as opool,
            tc.tile_pool(name="small", bufs=1) as small,
        ):
            # preload the ACT exp LUT off the critical path (the only ACT
            # table the kernel uses: sigmoid is computed via exp+reciprocal)
            warm = small.tile([1, 4], F32)
            nc.gpsimd.memset(warm[:], 0.0)
            warm2 = small.tile([1, 4], F32)
            nc.scalar.activation(warm2[:], warm[:], AF.Exp)

            # stage for the PE reshape: partition 0 carries the phase-1
            # results, rows 1-127 must be finite (they multiply e0's zeros).
            # f32r memset is invalid ISA, so zero an f32 twin and cast-copy.
            stage_z = small.tile([P, 3 * S], F32)
            nc.gpsimd.memset(stage_z[:], 0.0)
            stage = small.tile([P, 3 * S], MM_DTYPE)
            nc.vector.tensor_copy(stage[:], stage_z[:])

            # ALL small DMAs ride the SWDGE (gpsimd) ring: the SP HWDGE ring
            # carries only weight traffic, keeping its DMA-completion
            # semaphore lanes clean (shared lanes across rings were observed
            # to delay ow-chunk completion sems by ~15us)
            xvecs = const.tile([P, XVECS_W], MM_DTYPE)
            nc.gpsimd.dma_start(xvecs[:], xvecs_d[:])
            svecs = const.tile([P, SVECS_W], F32)
            nc.gpsimd.dma_start(svecs[:], svecs_d[:])

            # pinned 1MB weight tile for HAM-warming filler matmuls (the PE
            # cold-clocks to 1.2 GHz whenever its duty cycle drops; cold
            # matmuls then can't keep up with the 425 GB/s weight stream)
            filler = const.tile([P, KSUB, S], MM_DTYPE)
            nc.gpsimd.dma_start(filler[:], wr_d[:, 0:KSUB, :])

            # ---- phase 1: rr/kk/vv = W[sl,:] @ x? ---------------------------
            wdrams = [wr_d, wk_d, wv_d]
            xoffs = [OFF_XR, OFF_XK, OFF_XV]
            with tc.tile_pool(name="ps1", bufs=1, space="PSUM") as ps1:
                psums = [ps1.tile([1, S], F32, name=f"ps_{i}") for i in range(3)]
                fl_ps = ps1.tile([1, S], F32, name="fl_ps")

                def fill_mm(n):
                    for i in range(n):
                        nc.tensor.matmul(
                            fl_ps[:],
                            lhsT=xvecs[:, 0:1],
                            rhs=filler[:, i % KSUB, :],
                            start=True,
                            stop=True,
                        )

                # warm the PE through the first-chunk DMA latency
                fill_mm(20)
                for chunk in range(NCHUNK):
                    wtiles = []
                    for wi in range(3):
                        wt = wpool.tile([P, KSUB, S], MM_DTYPE, tag="wchunk")
                        nc.sync.dma_start(
                            wt[:], wdrams[wi][:, chunk * KSUB:(chunk + 1) * KSUB, :])
                        wtiles.append(wt)
                    for wi in range(3):
                        for tl in range(KSUB):
                            kt = chunk * KSUB + tl
                            nc.tensor.matmul(
                                psums[wi][:],
                                lhsT=xvecs[:, xoffs[wi] + kt:xoffs[wi] + kt + 1],
                                rhs=wtiles[wi][:, tl, :],
                                start=(kt == 0),
                                stop=(kt == KT - 1),
                            )
                        fill_mm(2)

                # ow weight DMAs emitted HERE so the in-order HWDGE ring plays
                # them right after the phase-1 weight DMAs; everything that
                # waits on phase-1 results stays off that ring.
                otiles = {}
                for tt in range(OW_KT):
                    for half in range(2):
                        ot = opool.tile([P, OW_HALF], MM_DTYPE, tag="owchunk")
                        nc.sync.dma_start(
                            ot[:], wo_d[tt][:, half * OW_HALF:(half + 1) * OW_HALF])
                        otiles[(tt, half)] = ot

                # PSUM -> SBUF (DMA cannot read PSUM; PE reads need SBUF)
                nc.scalar.copy(stage[0:1, 0:S], psums[0][:])
                nc.vector.tensor_copy(stage[0:1, S:2 * S], psums[1][:])
                nc.vector.tensor_copy(stage[0:1, 2 * S:3 * S], psums[2][:])

                # ---- reshape [1,1536] -> [128,12] on the PE ----------------
                # matmul(out, lhsT=stage[:, off:off+128], rhs=e0): out[p,0] =
                # sum_k stage[k, off+p]*e0[k] = stage[0, off+p] — a transpose
                # of a 128-wide row slice; also keeps PE HAM-warm through WKV.
                rs_ps = ps1.tile([P, 24], F32, name="rs_ps")
                e0 = xvecs[:, OFF_E0:OFF_E0 + 2]
                for v in range(3):
                    for j in range(OW_KT):
                        c2 = 2 * (v * 4 + j)
                        nc.tensor.matmul(
                            rs_ps[:, c2:c2 + 2],
                            lhsT=stage[:, v * S + j * P:v * S + (j + 1) * P],
                            rhs=e0,
                            start=True,
                            stop=True,
                        )
                rkv = small.tile([P, 12], F32)
                nc.vector.tensor_copy(rkv[:], rs_ps[:, 0:24:2])

            rr128 = rkv[:, 0:4]
            kk = rkv[:, 4:8]
            vv = rkv[:, 8:12]
            # r = sigmoid(rr) = 1 / (1 + exp(-rr)) — exp table only
            er = small.tile([P, 4], F32, name="er")
            nc.scalar.activation(er[:], rr128, AF.Exp, scale=-1.0)
            rp1 = small.tile([P, 4], F32, name="rp1")
            nc.vector.tensor_scalar_add(rp1[:], er[:], 1.0)
            r128 = small.tile([P, 4], F32, name="r128")
            nc.vector.reciprocal(r128[:], rp1[:])

            # ---- WKV recurrence (all [128, 4]) ------------------------------
            aa = svecs[:, OFF_AA:OFF_AA + 4]
            bb = svecs[:, OFF_BB:OFF_BB + 4]
            pp = svecs[:, OFF_PP:OFF_PP + 4]
            tf = svecs[:, OFF_TF:OFF_TF + 4]
            td = svecs[:, OFF_TD:OFF_TD + 4]

            def t4(name):
                return small.tile([P, 4], F32, name=name)

            nst = small.tile([P, 12], F32)
            na, nb_t, p2 = nst[:, 0:4], nst[:, 4:8], nst[:, 8:12]

            # output accumulators
            ww1 = t4("ww1")
            nc.vector.tensor_add(ww1, tf, kk[:])
            p1 = t4("p1")
            nc.vector.tensor_max(p1, pp, ww1)
            d1 = t4("d1")
            nc.vector.tensor_sub(d1, pp, p1)
            e1a = t4("e1a")
            nc.scalar.activation(e1a, d1, AF.Exp)
            d2 = t4("d2")
            nc.vector.tensor_sub(d2, ww1, p1)
            e2a = t4("e2a")
            nc.scalar.activation(e2a, d2, AF.Exp)
            acc_a = t4("acc_a")
            nc.vector.tensor_mul(acc_a, e1a, aa)
            tmp_a = t4("tmp_a")
            nc.vector.tensor_mul(tmp_a, e2a, vv[:])
            nc.vector.tensor_add(acc_a, acc_a, tmp_a)
            acc_b = t4("acc_b")
            nc.vector.tensor_mul(acc_b, e1a, bb)
            nc.vector.tensor_add(acc_b, acc_b, e2a)
            binv = t4("binv")
            nc.vector.reciprocal(binv, acc_b)
            y = t4("y")
            nc.vector.tensor_mul(y, acc_a, binv)   # wkv
            nc.vector.tensor_mul(y, r128[:], y)    # r * wkv

            # state update
            ww2 = t4("ww2")
            nc.vector.tensor_add(ww2, pp, td)
            nc.vector.tensor_max(p2, ww2, kk[:])
            d3 = t4("d3")
            nc.vector.tensor_sub(d3, ww2, p2)
            e1b = t4("e1b")
            nc.scalar.activation(e1b, d3, AF.Exp)
            d4 = t4("d4")
            nc.vector.tensor_sub(d4, kk[:], p2)
            e2b = t4("e2b")
            nc.scalar.activation(e2b, d4, AF.Exp)
            nc.vector.tensor_mul(na, e1b, aa)
            tmp_b = t4("tmp_b")
            nc.vector.tensor_mul(tmp_b, e2b, vv[:])
            nc.vector.tensor_add(na, na, tmp_b)
            nc.vector.tensor_mul(nb_t, e1b, bb)
            nc.vector.tensor_add(nb_t, nb_t, e2b)

            nc.gpsimd.dma_start(nst_d[:], nst[:])

            # round y to fp32r for the ow matmuls (verifier requirement)
            y_r = small.tile([P, 4], MM_DTYPE)
            nc.vector.tensor_copy(y_r[:], y[:])

            # ---- phase 2: partial = ow[:, sl] @ y ---------------------------
            with tc.tile_pool(name="ps2", bufs=1, space="PSUM") as ps2:
                ow_ps = ps2.tile([1, C], F32)
                out_sb = small.tile([1, C], F32)
                # dummy matmuls: keep the PE HAM-warm through the WKV gap so
                # the real ow matmuls run at 2.4 GHz; the first real bank-0
                # matmul (start=True) resets the bank
                for _ in range(6):
                    nc.tensor.matmul(
                        ow_ps[:, 0:512],
                        lhsT=xvecs[:, 0:1],
                        rhs=otiles[(0, 0)][:, 0:512],
                        start=True,
                        stop=True,
                    )
                for tt in range(OW_KT):
                    for nb in range(C // 512):
                        half, col = nb // 4, nb % 4
                        nc.tensor.matmul(
                            ow_ps[:, nb * 512:(nb + 1) * 512],
                            lhsT=y_r[:, tt:tt + 1],
                            rhs=otiles[(tt, half)][:, col * 512:(col + 1) * 512],
                            start=(tt == 0),
                            stop=(tt == OW_KT - 1),
                        )
                        if tt == OW_KT - 1:
                            # bank nb is complete; copy out while later banks
                            # are still accumulating
                            sl_ = slice(nb * 512, (nb + 1) * 512)
                            if nb % 2 == 0:
                                nc.vector.tensor_copy(out_sb[:, sl_], ow_ps[:, sl_])
                            else:
                                nc.scalar.copy(out_sb[:, sl_], ow_ps[:, sl_])
                            if nb == 3:
                                # first half of the output leaves while banks
                                # 4-7 are still accumulating
                                nc.gpsimd.dma_start(
                                    partial_d[:, 0:C // 2], out_sb[:, 0:C // 2])

            nc.gpsimd.dma_start(partial_d[:, C // 2:], out_sb[:, C // 2:])

    nc.compile()
    return nc


def _prep_in_maps(x, state, state_a, state_b, state_p,
                  time_mix_k, time_mix_v, time_mix_r,
                  time_first, time_decay, kw, vw, rw, ow):
    f = lambda a: np.ascontiguousarray(np.asarray(a), dtype=np.float32)
    x, state = f(x), f(state)
    tmk, tmv, tmr = f(time_mix_k), f(time_mix_v), f(time_mix_r)
    xk = (x * tmk + state * (1.0 - tmk)).reshape(P, KT)
    xv = (x * tmv + state * (1.0 - tmv)).reshape(P, KT)
    xr = (x * tmr + state * (1.0 - tmr)).reshape(P, KT)
    aa, bb, pp = f(state_a), f(state_b), f(state_p)
    tf, td = f(time_first), f(time_decay)
    kw, vw, rw, ow = f(kw), f(vw), f(rw), f(ow)

    xvecs = np.zeros((P, XVECS_W), dtype=np.float32)
    xvecs[:, OFF_XK:OFF_XK + KT] = xk
    xvecs[:, OFF_XV:OFF_XV + KT] = xv
    xvecs[:, OFF_XR:OFF_XR + KT] = xr
    xvecs[0, OFF_E0] = 1.0

    # WKV-side [128, 4] layout: channel = j*128 + p
    pm = lambda v: np.ascontiguousarray(v.reshape(OW_KT, P).T)
    in_maps = []
    for c in range(NCORES):
        sl = slice(c * S, (c + 1) * S)
        svecs = np.empty((P, SVECS_W), dtype=np.float32)
        svecs[:, OFF_AA:OFF_AA + 4] = pm(aa[sl])
        svecs[:, OFF_BB:OFF_BB + 4] = pm(bb[sl])
        svecs[:, OFF_PP:OFF_PP + 4] = pm(pp[sl])
        svecs[:, OFF_TF:OFF_TF + 4] = pm(tf[sl])
        svecs[:, OFF_TD:OFF_TD + 4] = pm(td[sl])
        in_maps.append({
            "xvecs": xvecs,
            "svecs": svecs,
            "wr": np.ascontiguousarray(rw[sl, :].T).reshape(P, KT, S),
            "wk": np.ascontiguousarray(kw[sl, :].T).reshape(P, KT, S),
            "wv": np.ascontiguousarray(vw[sl, :].T).reshape(P, KT, S),
            "wo": np.ascontiguousarray(ow[:, sl].T).reshape(OW_KT, P, C),
        })
    return in_maps, x


_NC_CACHE = None


def _run(inputs, trace=False):
    global _NC_CACHE
    if _NC_CACHE is None:
        _NC_CACHE = _build()
    nc = _NC_CACHE
    in_maps, x = _prep_in_maps(**inputs)
    res = bass_utils.run_bass_kernel_spmd(
        nc, in_maps, core_ids=list(range(NCORES)), trace=trace)

    out = np.zeros(C, dtype=np.float32)
    new_a = np.empty(C, dtype=np.float32)
    new_b = np.empty(C, dtype=np.float32)
    new_p = np.empty(C, dtype=np.float32)
    for c in range(NCORES):
        r = res.results[c]
        out += r["partial"].reshape(C)
        sl = slice(c * S, (c + 1) * S)
        nst = r["nst"]
        # [p, j] -> channel j*128 + p
        new_a[sl] = nst[:, 0:4].T.reshape(S)
        new_b[sl] = nst[:, 4:8].T.reshape(S)
        new_p[sl] = nst[:, 8:12].T.reshape(S)
    return (out, x.copy(), new_a, new_b, new_p), res


def kernel(**inputs):
    outs, _ = _run(inputs, trace=False)
    return outs



# revision 2
# speedup vs baseline: 1.4350x; 1.4350x over previous
"""RWKV time-mixing (C=4096) on 8 trn2 NeuronCores.

Strategy (tensor-parallel over channels, M=8 cores, S=C/M=512):
  - Core c owns channels sl = [c*512, (c+1)*512).
  - Weights stream in bf16 (matvec is HBM-bound; bf16 halves the traffic
    to 16 MB/core and the rel-err budget of 2e-2 has ~5x margin over the
    measured bf16 error). All elementwise WKV math stays fp32, and the
    PE-reshape trick stays fp32r so kk keeps full precision into exp().
  - Phase 1: kk/vv/rr shards: kw[sl,:] @ xk etc. Weights are host-transposed
    so the contraction dim lands on SBUF partitions; the x-vector column is
    the (tiny) stationary operand, weight tiles stream as the moving operand
    at N=512 in bf16 (1 cycle/row on the PE).
  - WKV recurrence: purely elementwise on the 512-channel shard, done in a
    [128, 4] layout (channel = j*128 + p).
  - Phase 2: partial out = ow[:, sl] @ (r*wkv): 4 k-tiles x 8 n-banks of
    matmuls into a [1, 4096] PSUM row; host sums the 8 per-core partials
    (the "all-reduce" of the column-sharded matvec).
  - new_state = x exactly (token shift), assembled on host.

k-index convention (phase 1): k = p*32 + n  (p = partition, n = k-tile id),
so W.T.reshape(128, 32, 512) puts k-tile n at [:, n, :] with contraction on
partitions, and x.reshape(128, 32) column n is the matching stationary vec.

The [1,512] -> [128,4] reshape of the phase-1 results runs on the PE: a
matmul whose stationary operand is a zero-padded [128,128] slice with the
data in partition 0 and whose moving operand is the unit vector e0 emits
the row slice as a [128,2] PSUM column pair (channel = j*128 + p). Phase 2
contracts k-tile tt=j over rows [j*128,(j+1)*128) of ow[:, sl].T, matching
that layout. This path is fp32r end-to-end: rounding kk to bf16 here would
put ~1% error into exp(kk) on the largest channels.
"""

import ml_dtypes
import numpy as np

import concourse.bass as bass
import concourse.mybir as mybir
import concourse.tile as tile
from concourse import bacc, bass_utils

C = 4096
NCORES = 8
S = C // NCORES          # 512 channels per core
P = 128
KT = C // P              # 32 k-tiles in phase 1
KSUB = 8                 # k-tiles per DMA chunk (1 MB bf16 chunks)
NCHUNK = KT // KSUB      # 4 chunks per phase-1 matrix
OW_KT = S // P           # 4 k-tiles in phase 2

F32 = mybir.dt.float32
F32R = mybir.dt.float32r
BF16 = mybir.dt.bfloat16
AF = mybir.ActivationFunctionType
MM_DTYPE = BF16          # weight-stream dtype: 1 cycle/row, 2 bytes/elem
NP_BF16 = ml_dtypes.bfloat16

# xvecs layout: [128, 96] = xk[0:32] | xv[32:64] | xr[64:96] (bf16)
# e0 (fp32r) is a separate [128, 2] input: unit vector [1,0,...,0] + a zero
# column (N=2 moving operand of the PE-reshape matmuls — fp32r matmuls
# require an even moving free size)
# svecs layout: [128, 20] = aa[0:4] | bb[4:8] | pp[8:12] | tf[12:16] | td[16:20]
XVECS_W = 3 * KT
SVECS_W = 5 * 4
OFF_XK, OFF_XV, OFF_XR = 0, KT, 2 * KT
OFF_AA, OFF_BB, OFF_PP, OFF_TF, OFF_TD = 0, 4, 8, 12, 16


def _build():
    nc = bacc.Bacc("TRN2", target_bir_lowering=False, debug=False,
                   num_devices=NCORES)

    xvecs_d = nc.dram_tensor("xvecs", [P, XVECS_W], MM_DTYPE, kind="ExternalInput")
    e0_d = nc.dram_tensor("e0", [P, 2], F32R, kind="ExternalInput")
    svecs_d = nc.dram_tensor("svecs", [P, SVECS_W], F32, kind="ExternalInput")
    wr_d = nc.dram_tensor("wr", [P, KT, S], MM_DTYPE, kind="ExternalInput")
    wk_d = nc.dram_tensor("wk", [P, KT, S], MM_DTYPE, kind="ExternalInput")
    wv_d = nc.dram_tensor("wv", [P, KT, S], MM_DTYPE, kind="ExternalInput")
    wo_d = nc.dram_tensor("wo", [OW_KT, P, C], MM_DTYPE, kind="ExternalInput")

    partial_d = nc.dram_tensor("partial", [1, C], F32, kind="ExternalOutput")
    nst_d = nc.dram_tensor("nst", [P, 12], F32, kind="ExternalOutput")

    with tile.TileContext(nc) as tc:
        with (
            tc.tile_pool(name="const", bufs=1) as const,
            tc.tile_pool(name="w", bufs=12) as wpool,
            tc.tile_pool(name="ow", bufs=4) as opool,
            tc.tile_pool(name="small", bufs=1) as small,
        ):
            # preload the ACT exp LUT off the critical path (the only ACT
            # table the kernel uses: sigmoid is computed via exp+reciprocal)
            warm = small.tile([1, 4], F32)
            nc.gpsimd.memset(warm[:], 0.0)
            warm2 = small.tile([1, 4], F32)
            nc.scalar.activation(warm2[:], warm[:], AF.Exp)

            # stage for the PE reshape: partition 0 carries the phase-1
            # results, rows 1-127 must be finite (they multiply e0's zeros).
            # f32r memset is invalid ISA, so zero an f32 twin and cast-copy.
            stage_z = small.tile([P, 3 * S], F32)
            nc.gpsimd.memset(stage_z[:], 0.0)
            stage = small.tile([P, 3 * S], F32R)
            nc.vector.tensor_copy(stage[:], stage_z[:])

            # ALL small DMAs ride the SWDGE (gpsimd) ring: the SP HWDGE ring
            # carries only weight traffic, keeping its DMA-completion
            # semaphore lanes clean (shared lanes across rings were observed
            # to delay ow-chunk completion sems by ~15us)
            xvecs = const.tile([P, XVECS_W], MM_DTYPE)
            nc.gpsimd.dma_start(xvecs[:], xvecs_d[:])
            e0 = const.tile([P, 2], F32R)
            nc.gpsimd.dma_start(e0[:], e0_d[:])
            svecs = const.tile([P, SVECS_W], F32)
            nc.gpsimd.dma_start(svecs[:], svecs_d[:])

            # pinned weight tile for HAM-warming filler matmuls (the PE
            # cold-clocks to 1.2 GHz whenever its duty cycle drops; cold
            # matmuls then can't keep up with the weight stream)
            filler = const.tile([P, 1, S], MM_DTYPE)
            nc.gpsimd.dma_start(filler[:], wr_d[:, 0:1, :])

            # ---- phase 1: rr/kk/vv = W[sl,:] @ x? ---------------------------
            wdrams = [wr_d, wk_d, wv_d]
            xoffs = [OFF_XR, OFF_XK, OFF_XV]
            with tc.tile_pool(name="ps1", bufs=1, space="PSUM") as ps1:
                psums = [ps1.tile([1, S], F32, name=f"ps_{i}") for i in range(3)]
                fl_ps = ps1.tile([1, S], F32, name="fl_ps")

                def fill_mm(n):
                    for i in range(n):
                        nc.tensor.matmul(
                            fl_ps[:],
                            lhsT=xvecs[:, 0:1],
                            rhs=filler[:, 0, :],
                            start=True,
                            stop=True,
                        )

                # warm the PE through the first-chunk DMA latency
                fill_mm(20)
                for chunk in range(NCHUNK):
                    wtiles = []
                    for wi in range(3):
                        wt = wpool.tile([P, KSUB, S], MM_DTYPE, tag="wchunk")
                        nc.sync.dma_start(
                            wt[:], wdrams[wi][:, chunk * KSUB:(chunk + 1) * KSUB, :])
                        wtiles.append(wt)
                    for wi in range(3):
                        for tl in range(KSUB):
                            kt = chunk * KSUB + tl
                            nc.tensor.matmul(
                                psums[wi][:],
                                lhsT=xvecs[:, xoffs[wi] + kt:xoffs[wi] + kt + 1],
                                rhs=wtiles[wi][:, tl, :],
                                start=(kt == 0),
                                stop=(kt == KT - 1),
                            )
                        fill_mm(2)

                # ow weight DMAs emitted HERE so the in-order HWDGE ring plays
                # them right after the phase-1 weight DMAs; everything that
                # waits on phase-1 results stays off that ring.
                otiles = []
                for tt in range(OW_KT):
                    ot = opool.tile([P, C], MM_DTYPE, tag="owchunk")
                    nc.sync.dma_start(ot[:], wo_d[tt][:, :])
                    otiles.append(ot)

                # PSUM -> SBUF (DMA cannot read PSUM; PE reads need SBUF)
                nc.scalar.copy(stage[0:1, 0:S], psums[0][:])
                nc.vector.tensor_copy(stage[0:1, S:2 * S], psums[1][:])
                nc.vector.tensor_copy(stage[0:1, 2 * S:3 * S], psums[2][:])

                # ---- reshape [1,1536] -> [128,12] on the PE ----------------
                # matmul(out, lhsT=stage[:, off:off+128], rhs=e0): out[p,0] =
                # sum_k stage[k, off+p]*e0[k] = stage[0, off+p] — a transpose
                # of a 128-wide row slice; also keeps PE HAM-warm through WKV.
                rs_ps = ps1.tile([P, 24], F32, name="rs_ps")
                for v in range(3):
                    for j in range(OW_KT):
                        c2 = 2 * (v * 4 + j)
                        nc.tensor.matmul(
                            rs_ps[:, c2:c2 + 2],
                            lhsT=stage[:, v * S + j * P:v * S + (j + 1) * P],
                            rhs=e0[:],
                            start=True,
                            stop=True,
                        )
                rkv = small.tile([P, 12], F32)
                nc.vector.tensor_copy(rkv[:], rs_ps[:, 0:24:2])

            rr128 = rkv[:, 0:4]
            kk = rkv[:, 4:8]
            vv = rkv[:, 8:12]
            # r = sigmoid(rr) = 1 / (1 + exp(-rr)) — exp table only
            er = small.tile([P, 4], F32, name="er")
            nc.scalar.activation(er[:], rr128, AF.Exp, scale=-1.0)
            rp1 = small.tile([P, 4], F32, name="rp1")
            nc.vector.tensor_scalar_add(rp1[:], er[:], 1.0)
            r128 = small.tile([P, 4], F32, name="r128")
            nc.vector.reciprocal(r128[:], rp1[:])

            # ---- WKV recurrence (all [128, 4]) ------------------------------
            aa = svecs[:, OFF_AA:OFF_AA + 4]
            bb = svecs[:, OFF_BB:OFF_BB + 4]
            pp = svecs[:, OFF_PP:OFF_PP + 4]
            tf = svecs[:, OFF_TF:OFF_TF + 4]
            td = svecs[:, OFF_TD:OFF_TD + 4]

            def t4(name):
                return small.tile([P, 4], F32, name=name)

            nst = small.tile([P, 12], F32)
            na, nb_t, p2 = nst[:, 0:4], nst[:, 4:8], nst[:, 8:12]

            # output accumulators
            ww1 = t4("ww1")
            nc.vector.tensor_add(ww1, tf, kk[:])
            p1 = t4("p1")
            nc.vector.tensor_max(p1, pp, ww1)
            d1 = t4("d1")
            nc.vector.tensor_sub(d1, pp, p1)
            e1a = t4("e1a")
            nc.scalar.activation(e1a, d1, AF.Exp)
            d2 = t4("d2")
            nc.vector.tensor_sub(d2, ww1, p1)
            e2a = t4("e2a")
            nc.scalar.activation(e2a, d2, AF.Exp)
            acc_a = t4("acc_a")
            nc.vector.tensor_mul(acc_a, e1a, aa)
            tmp_a = t4("tmp_a")
            nc.vector.tensor_mul(tmp_a, e2a, vv[:])
            nc.vector.tensor_add(acc_a, acc_a, tmp_a)
            acc_b = t4("acc_b")
            nc.vector.tensor_mul(acc_b, e1a, bb)
            nc.vector.tensor_add(acc_b, acc_b, e2a)
            binv = t4("binv")
            nc.vector.reciprocal(binv, acc_b)
            y = t4("y")
            nc.vector.tensor_mul(y, acc_a, binv)   # wkv
            nc.vector.tensor_mul(y, r128[:], y)    # r * wkv

            # state update
            ww2 = t4("ww2")
            nc.vector.tensor_add(ww2, pp, td)
            nc.vector.tensor_max(p2, ww2, kk[:])
            d3 = t4("d3")
            nc.vector.tensor_sub(d3, ww2, p2)
            e1b = t4("e1b")
            nc.scalar.activation(e1b, d3, AF.Exp)
            d4 = t4("d4")
            nc.vector.tensor_sub(d4, kk[:], p2)
            e2b = t4("e2b")
            nc.scalar.activation(e2b, d4, AF.Exp)
            nc.vector.tensor_mul(na, e1b, aa)
            tmp_b = t4("tmp_b")
            nc.vector.tensor_mul(tmp_b, e2b, vv[:])
            nc.vector.tensor_add(na, na, tmp_b)
            nc.vector.tensor_mul(nb_t, e1b, bb)
            nc.vector.tensor_add(nb_t, nb_t, e2b)

            nc.gpsimd.dma_start(nst_d[:], nst[:])

            # round y to bf16 for the ow matmuls (operand dtypes must match)
            y_r = small.tile([P, 4], MM_DTYPE)
            nc.vector.tensor_copy(y_r[:], y[:])

            # ---- phase 2: partial = ow[:, sl] @ y ---------------------------
            with tc.tile_pool(name="ps2", bufs=1, space="PSUM") as ps2:
                ow_ps = ps2.tile([1, C], F32)
                out_sb = small.tile([1, C], F32)
                # dummy matmuls: keep the PE HAM-warm through the WKV gap so
                # the real ow matmuls run at 2.4 GHz; the first real bank-0
                # matmul (start=True) resets the bank
                for _ in range(6):
                    nc.tensor.matmul(
                        ow_ps[:, 0:512],
                        lhsT=xvecs[:, 0:1],
                        rhs=otiles[0][:, 0:512],
                        start=True,
                        stop=True,
                    )
                for tt in range(OW_KT):
                    for nb in range(C // 512):
                        nc.tensor.matmul(
                            ow_ps[:, nb * 512:(nb + 1) * 512],
                            lhsT=y_r[:, tt:tt + 1],
                            rhs=otiles[tt][:, nb * 512:(nb + 1) * 512],
                            start=(tt == 0),
                            stop=(tt == OW_KT - 1),
                        )
                        if tt == OW_KT - 1:
                            # bank nb is complete; copy out while later banks
                            # are still accumulating
                            sl_ = slice(nb * 512, (nb + 1) * 512)
                            if nb % 2 == 0:
                                nc.vector.tensor_copy(out_sb[:, sl_], ow_ps[:, sl_])
                            else:
                                nc.scalar.copy(out_sb[:, sl_], ow_ps[:, sl_])
                            if nb == 3:
                                # first half of the output leaves while banks
                                # 4-7 are still accumulating
                                nc.gpsimd.dma_start(
                                    partial_d[:, 0:C // 2], out_sb[:, 0:C // 2])

            nc.gpsimd.dma_start(partial_d[:, C // 2:], out_sb[:, C // 2:])

    nc.compile()
    return nc


def _prep_in_maps(x, state, state_a, state_b, state_p,
                  time_mix_k, time_mix_v, time_mix_r,
                  time_first, time_decay, kw, vw, rw, ow):
    f = lambda a: np.ascontiguousarray(np.asarray(a), dtype=np.float32)
    x, state = f(x), f(state)
    tmk, tmv, tmr = f(time_mix_k), f(time_mix_v), f(time_mix_r)
    xk = (x * tmk + state * (1.0 - tmk)).reshape(P, KT)
    xv = (x * tmv + state * (1.0 - tmv)).reshape(P, KT)
    xr = (x * tmr + state * (1.0 - tmr)).reshape(P, KT)
    aa, bb, pp = f(state_a), f(state_b), f(state_p)
    tf, td = f(time_first), f(time_decay)
    kw, vw, rw, ow = f(kw), f(vw), f(rw), f(ow)

    xvecs = np.zeros((P, XVECS_W), dtype=np.float32)
    xvecs[:, OFF_XK:OFF_XK + KT] = xk
    xvecs[:, OFF_XV:OFF_XV + KT] = xv
    xvecs[:, OFF_XR:OFF_XR + KT] = xr
    xvecs = xvecs.astype(NP_BF16)

    e0 = np.zeros((P, 2), dtype=np.float32)
    e0[0, 0] = 1.0

    wb = lambda a: np.ascontiguousarray(a).astype(NP_BF16)

    # WKV-side [128, 4] layout: channel = j*128 + p
    pm = lambda v: np.ascontiguousarray(v.reshape(OW_KT, P).T)
    in_maps = []
    for c in range(NCORES):
        sl = slice(c * S, (c + 1) * S)
        svecs = np.empty((P, SVECS_W), dtype=np.float32)
        svecs[:, OFF_AA:OFF_AA + 4] = pm(aa[sl])
        svecs[:, OFF_BB:OFF_BB + 4] = pm(bb[sl])
        svecs[:, OFF_PP:OFF_PP + 4] = pm(pp[sl])
        svecs[:, OFF_TF:OFF_TF + 4] = pm(tf[sl])
        svecs[:, OFF_TD:OFF_TD + 4] = pm(td[sl])
        in_maps.append({
            "xvecs": xvecs,
            "e0": e0,
            "svecs": svecs,
            "wr": wb(rw[sl, :].T).reshape(P, KT, S),
            "wk": wb(kw[sl, :].T).reshape(P, KT, S),
            "wv": wb(vw[sl, :].T).reshape(P, KT, S),
            "wo": wb(ow[:, sl].T).reshape(OW_KT, P, C),
        })
    return in_maps, x


_NC_CACHE = None


def _run(inputs, trace=False):
    global _NC_CACHE
    if _NC_CACHE is None:
        _NC_CACHE = _build()
    nc = _NC_CACHE
    in_maps, x = _prep_in_maps(**inputs)
    res = bass_utils.run_bass_kernel_spmd(
        nc, in_maps, core_ids=list(range(NCORES)), trace=trace)

    out = np.zeros(C, dtype=np.float32)
    new_a = np.empty(C, dtype=np.float32)
    new_b = np.empty(C, dtype=np.float32)
    new_p = np.empty(C, dtype=np.float32)
    for c in range(NCORES):
        r = res.results[c]
        out += r["partial"].reshape(C)
        sl = slice(c * S, (c + 1) * S)
        nst = r["nst"]
        # [p, j] -> channel j*128 + p
        new_a[sl] = nst[:, 0:4].T.reshape(S)
        new_b[sl] = nst[:, 4:8].T.reshape(S)
        new_p[sl] = nst[:, 8:12].T.reshape(S)
    return (out, x.copy(), new_a, new_b, new_p), res


def kernel(**inputs):
    outs, _ = _run(inputs, trace=False)
    return outs


# revision 3
# speedup vs baseline: 1.4937x; 1.0409x over previous
"""RWKV time-mixing (C=4096) on 8 trn2 NeuronCores.

Strategy (tensor-parallel over channels, M=8 cores, S=C/M=512):
  - Core c owns channels sl = [c*512, (c+1)*512).
  - Weights stream in bf16 (matvec is HBM-bound; bf16 halves the traffic
    to 16 MB/core and the rel-err budget of 2e-2 has ~6x margin over the
    measured bf16 error). All elementwise WKV math stays fp32, and the
    PE-reshape trick stays fp32r so kk keeps full precision into exp().
  - Phase 1: kk/vv/rr shards: kw[sl,:] @ xk etc. Weights are host-transposed
    so the contraction dim lands on SBUF partitions; the x-vector column is
    the (tiny) stationary operand, weight tiles stream as the moving operand
    at N=512 in bf16 (1 cycle/row on the PE).
  - WKV recurrence: purely elementwise on the 512-channel shard, done in a
    [128, 4] layout (channel = j*128 + p).
  - Phase 2: partial out = ow[:, sl] @ (r*wkv): 4 k-tiles x 8 n-banks of
    matmuls into a [1, 4096] PSUM row; host sums the 8 per-core partials
    (the "all-reduce" of the column-sharded matvec).
  - new_state = x exactly (token shift), assembled on host.

Stream order is MATRIX-MAJOR (all of rw, then kw, then vw, then ow) so the
dependent chain drains while later weights stream: rr is complete ~1/4 into
the stream (sigmoid runs there), kk at ~2/4 (the kk-only part of the WKV
recurrence runs there), vv at ~3/4 (the short vv tail + y = r*wkv run
there, just before the first ow chunk lands), and the phase-2 matmuls then
chase the ow chunks as they arrive. Only the last ow half-chunk's four
matmuls + PSUM copies + output DMA remain after the final weight byte.

k-index convention (phase 1): k = p*32 + n  (p = partition, n = k-tile id),
so W.T.reshape(128, 32, 512) puts k-tile n at [:, n, :] with contraction on
partitions, and x.reshape(128, 32) column n is the matching stationary vec.

The [1,512] -> [128,4] reshape of the phase-1 results runs on the PE: a
matmul whose stationary operand is a zero-padded [128,128] slice with the
data in partition 0 and whose moving operand is the unit vector e0 emits
the row slice as a [128,2] PSUM column pair (channel = j*128 + p). Phase 2
contracts k-tile tt=j over rows [j*128,(j+1)*128) of ow[:, sl].T, matching
that layout. This path is fp32r end-to-end: rounding kk to bf16 here would
put ~1% error into exp(kk) on the largest channels.
"""

import ml_dtypes
import numpy as np

import concourse.bass as bass
import concourse.mybir as mybir
import concourse.tile as tile
from concourse import bacc, bass_utils

C = 4096
NCORES = 8
S = C // NCORES          # 512 channels per core
P = 128
KT = C // P              # 32 k-tiles in phase 1
KSUB = 8                 # k-tiles per DMA chunk (1 MB bf16 chunks)
NCHUNK = KT // KSUB      # 4 chunks per phase-1 matrix
OW_KT = S // P           # 4 k-tiles in phase 2
OW_HALF = C // 2         # ow k-tile chunks split into 512 KB column halves

F32 = mybir.dt.float32
F32R = mybir.dt.float32r
BF16 = mybir.dt.bfloat16
AF = mybir.ActivationFunctionType
MM_DTYPE = BF16          # weight-stream dtype: 1 cycle/row, 2 bytes/elem
NP_BF16 = ml_dtypes.bfloat16

# xvecs layout: [128, 96] = xk[0:32] | xv[32:64] | xr[64:96] (bf16)
# e0 (fp32r) is a separate [128, 2] input: unit vector [1,0,...,0] + a zero
# column (N=2 moving operand of the PE-reshape matmuls — fp32r matmuls
# require an even moving free size)
# svecs layout: [128, 20] = aa[0:4] | bb[4:8] | pp[8:12] | tf[12:16] | td[16:20]
XVECS_W = 3 * KT
SVECS_W = 5 * 4
OFF_XK, OFF_XV, OFF_XR = 0, KT, 2 * KT
OFF_AA, OFF_BB, OFF_PP, OFF_TF, OFF_TD = 0, 4, 8, 12, 16


def _build():
    nc = bacc.Bacc("TRN2", target_bir_lowering=False, debug=False,
                   num_devices=NCORES)

    xvecs_d = nc.dram_tensor("xvecs", [P, XVECS_W], MM_DTYPE, kind="ExternalInput")
    e0_d = nc.dram_tensor("e0", [P, 2], F32R, kind="ExternalInput")
    svecs_d = nc.dram_tensor("svecs", [P, SVECS_W], F32, kind="ExternalInput")
    wr_d = nc.dram_tensor("wr", [P, KT, S], MM_DTYPE, kind="ExternalInput")
    wk_d = nc.dram_tensor("wk", [P, KT, S], MM_DTYPE, kind="ExternalInput")
    wv_d = nc.dram_tensor("wv", [P, KT, S], MM_DTYPE, kind="ExternalInput")
    wo_d = nc.dram_tensor("wo", [OW_KT, P, C], MM_DTYPE, kind="ExternalInput")

    partial_d = nc.dram_tensor("partial", [1, C], F32, kind="ExternalOutput")
    nst_d = nc.dram_tensor("nst", [P, 12], F32, kind="ExternalOutput")

    with tile.TileContext(nc) as tc:
        with (
            tc.tile_pool(name="const", bufs=1) as const,
            tc.tile_pool(name="w", bufs=12) as wpool,
            tc.tile_pool(name="ow", bufs=8) as opool,
            tc.tile_pool(name="small", bufs=1) as small,
        ):
            # preload the ACT exp LUT off the critical path (the only ACT
            # table the kernel uses: sigmoid is computed via exp+reciprocal)
            warm = small.tile([1, 4], F32)
            nc.gpsimd.memset(warm[:], 0.0)
            warm2 = small.tile([1, 4], F32)
            nc.scalar.activation(warm2[:], warm[:], AF.Exp)

            # stage for the PE reshape: partition 0 carries the phase-1
            # results, rows 1-127 must be finite (they multiply e0's zeros).
            # f32r memset is invalid ISA, so zero an f32 twin and cast-copy.
            stage_z = small.tile([P, 3 * S], F32)
            nc.gpsimd.memset(stage_z[:], 0.0)
            stage = small.tile([P, 3 * S], F32R)
            nc.vector.tensor_copy(stage[:], stage_z[:])

            # ALL small DMAs ride the SWDGE (gpsimd) ring: the SP HWDGE ring
            # carries only weight traffic, keeping its DMA-completion
            # semaphore lanes clean (shared lanes across rings were observed
            # to delay ow-chunk completion sems by ~15us)
            xvecs = const.tile([P, XVECS_W], MM_DTYPE)
            nc.gpsimd.dma_start(xvecs[:], xvecs_d[:])
            e0 = const.tile([P, 2], F32R)
            nc.gpsimd.dma_start(e0[:], e0_d[:])
            svecs = const.tile([P, SVECS_W], F32)
            nc.gpsimd.dma_start(svecs[:], svecs_d[:])

            # pinned weight tile for HAM-warming filler matmuls (the PE
            # cold-clocks to 1.2 GHz whenever its duty cycle drops; cold
            # matmuls then can't keep up with the weight stream)
            filler = const.tile([P, 1, S], MM_DTYPE)
            nc.gpsimd.dma_start(filler[:], wr_d[:, 0:1, :])

            aa = svecs[:, OFF_AA:OFF_AA + 4]
            bb = svecs[:, OFF_BB:OFF_BB + 4]
            pp = svecs[:, OFF_PP:OFF_PP + 4]
            tf = svecs[:, OFF_TF:OFF_TF + 4]
            td = svecs[:, OFF_TD:OFF_TD + 4]

            def t4(name):
                return small.tile([P, 4], F32, name=name)

            nst = small.tile([P, 12], F32)
            na, nb_t, p2 = nst[:, 0:4], nst[:, 4:8], nst[:, 8:12]
            rkv = small.tile([P, 12], F32)
            rr128 = rkv[:, 0:4]
            kk = rkv[:, 4:8]
            vv = rkv[:, 8:12]

            # ---- phase 1 + overlapped recurrence ---------------------------
            wdrams = [wr_d, wk_d, wv_d]
            xoffs = [OFF_XR, OFF_XK, OFF_XV]
            # WKV temporaries shared between the kk-stage and the vv-stage
            r128 = small.tile([P, 4], F32, name="r128")
            e2a = t4("e2a")
            e2b = t4("e2b")
            acc_a = t4("acc_a")
            binv = t4("binv")
            y = t4("y")

            with tc.tile_pool(name="ps1", bufs=1, space="PSUM") as ps1:
                psums = [ps1.tile([1, S], F32, name=f"ps_{i}") for i in range(3)]
                fl_ps = ps1.tile([1, S], F32, name="fl_ps")
                rs_ps = ps1.tile([P, 24], F32, name="rs_ps")

                def fill_mm(n):
                    for i in range(n):
                        nc.tensor.matmul(
                            fl_ps[:],
                            lhsT=xvecs[:, 0:1],
                            rhs=filler[:, 0, :],
                            start=True,
                            stop=True,
                        )

                # warm the PE through the first-chunk DMA latency
                fill_mm(20)
                for wi in range(3):
                    for chunk in range(NCHUNK):
                        wt = wpool.tile([P, KSUB, S], MM_DTYPE, tag="wchunk")
                        nc.sync.dma_start(
                            wt[:], wdrams[wi][:, chunk * KSUB:(chunk + 1) * KSUB, :])
                        for tl in range(KSUB):
                            kt = chunk * KSUB + tl
                            nc.tensor.matmul(
                                psums[wi][:],
                                lhsT=xvecs[:, xoffs[wi] + kt:xoffs[wi] + kt + 1],
                                rhs=wt[:, tl, :],
                                start=(kt == 0),
                                stop=(kt == KT - 1),
                            )
                        fill_mm(1)

                    # matrix wi fully reduced: transpose its [1,512] row into
                    # the [128,4] WKV layout while the next matrix streams
                    if wi == 0:
                        nc.scalar.copy(stage[0:1, 0:S], psums[0][:])
                    else:
                        nc.vector.tensor_copy(
                            stage[0:1, wi * S:(wi + 1) * S], psums[wi][:])
                    for j in range(OW_KT):
                        c2 = 2 * (wi * 4 + j)
                        nc.tensor.matmul(
                            rs_ps[:, c2:c2 + 2],
                            lhsT=stage[:, wi * S + j * P:wi * S + (j + 1) * P],
                            rhs=e0[:],
                            start=True,
                            stop=True,
                        )
                    nc.vector.tensor_copy(
                        rkv[:, wi * 4:wi * 4 + 4],
                        rs_ps[:, 2 * wi * 4:2 * wi * 4 + 8:2])

                    if wi == 0:
                        # r = sigmoid(rr) = 1 / (1 + exp(-rr)) — exp only
                        er = t4("er")
                        nc.scalar.activation(er[:], rr128, AF.Exp, scale=-1.0)
                        rp1 = t4("rp1")
                        nc.vector.tensor_scalar_add(rp1[:], er[:], 1.0)
                        nc.vector.reciprocal(r128[:], rp1[:])
                    elif wi == 1:
                        # everything in the WKV recurrence that needs only kk
                        ww1 = t4("ww1")
                        nc.vector.tensor_add(ww1, tf, kk[:])
                        p1 = t4("p1")
                        nc.vector.tensor_max(p1, pp, ww1)
                        d1 = t4("d1")
                        nc.vector.tensor_sub(d1, pp, p1)
                        e1a = t4("e1a")
                        nc.scalar.activation(e1a, d1, AF.Exp)
                        d2 = t4("d2")
                        nc.vector.tensor_sub(d2, ww1, p1)
                        nc.scalar.activation(e2a[:], d2, AF.Exp)
                        nc.vector.tensor_mul(acc_a[:], e1a, aa)   # e1*aa
                        acc_b = t4("acc_b")
                        nc.vector.tensor_mul(acc_b, e1a, bb)
                        nc.vector.tensor_add(acc_b, acc_b, e2a[:])
                        nc.vector.reciprocal(binv[:], acc_b)
                        # state update, kk-only part
                        ww2 = t4("ww2")
                        nc.vector.tensor_add(ww2, pp, td)
                        nc.vector.tensor_max(p2, ww2, kk[:])
                        d3 = t4("d3")
                        nc.vector.tensor_sub(d3, ww2, p2)
                        e1b = t4("e1b")
                        nc.scalar.activation(e1b, d3, AF.Exp)
                        d4 = t4("d4")
                        nc.vector.tensor_sub(d4, kk[:], p2)
                        nc.scalar.activation(e2b[:], d4, AF.Exp)
                        nc.vector.tensor_mul(na, e1b, aa)         # e1*aa
                        nc.vector.tensor_mul(nb_t, e1b, bb)
                        nc.vector.tensor_add(nb_t, nb_t, e2b[:])
                    else:
                        # short vv tail: a = e1*aa + e2*vv, y = r * a/b
                        tmp_a = t4("tmp_a")
                        nc.vector.tensor_mul(tmp_a, e2a[:], vv[:])
                        nc.vector.tensor_add(acc_a[:], acc_a[:], tmp_a)
                        nc.vector.tensor_mul(y[:], acc_a[:], binv[:])  # wkv
                        nc.vector.tensor_mul(y[:], r128[:], y[:])      # r*wkv
                        tmp_b = t4("tmp_b")
                        nc.vector.tensor_mul(tmp_b, e2b[:], vv[:])
                        nc.vector.tensor_add(na, na, tmp_b)

                # ow weight DMAs: the in-order HWDGE ring plays them right
                # after the phase-1 weight DMAs, by which time y is ready and
                # the phase-2 matmuls chase the arriving chunks.
                otiles = {}
                for tt in range(OW_KT):
                    for half in range(2):
                        ot = opool.tile([P, OW_HALF], MM_DTYPE, tag="owchunk")
                        nc.sync.dma_start(
                            ot[:], wo_d[tt][:, half * OW_HALF:(half + 1) * OW_HALF])
                        otiles[(tt, half)] = ot

            nc.gpsimd.dma_start(nst_d[:], nst[:])

            # round y to bf16 for the ow matmuls (operand dtypes must match)
            y_r = small.tile([P, 4], MM_DTYPE)
            nc.vector.tensor_copy(y_r[:], y[:])

            # ---- phase 2: partial = ow[:, sl] @ y ---------------------------
            with tc.tile_pool(name="ps2", bufs=1, space="PSUM") as ps2:
                ow_ps = ps2.tile([1, C], F32)
                out_sb = small.tile([1, C], F32)
                for tt in range(OW_KT):
                    for nb in range(C // 512):
                        half, col = nb // 4, nb % 4
                        nc.tensor.matmul(
                            ow_ps[:, nb * 512:(nb + 1) * 512],
                            lhsT=y_r[:, tt:tt + 1],
                            rhs=otiles[(tt, half)][:, col * 512:(col + 1) * 512],
                            start=(tt == 0),
                            stop=(tt == OW_KT - 1),
                        )
                        if tt == OW_KT - 1:
                            # bank nb is complete; copy out while later banks
                            # are still accumulating
                            sl_ = slice(nb * 512, (nb + 1) * 512)
                            if nb % 2 == 0:
                                nc.vector.tensor_copy(out_sb[:, sl_], ow_ps[:, sl_])
                            else:
                                nc.scalar.copy(out_sb[:, sl_], ow_ps[:, sl_])
                            if nb == 3:
                                # first half of the output leaves while banks
                                # 4-7 are still accumulating
                                nc.gpsimd.dma_start(
                                    partial_d[:, 0:C // 2], out_sb[:, 0:C // 2])

            nc.gpsimd.dma_start(partial_d[:, C // 2:], out_sb[:, C // 2:])

    nc.compile()
    return nc


def _prep_in_maps(x, state, state_a, state_b, state_p,
                  time_mix_k, time_mix_v, time_mix_r,
                  time_first, time_decay, kw, vw, rw, ow):
    f = lambda a: np.ascontiguousarray(np.asarray(a), dtype=np.float32)
    x, state = f(x), f(state)
    tmk, tmv, tmr = f(time_mix_k), f(time_mix_v), f(time_mix_r)
    xk = (x * tmk + state * (1.0 - tmk)).reshape(P, KT)
    xv = (x * tmv + state * (1.0 - tmv)).reshape(P, KT)
    xr = (x * tmr + state * (1.0 - tmr)).reshape(P, KT)
    aa, bb, pp = f(state_a), f(state_b), f(state_p)
    tf, td = f(time_first), f(time_decay)
    kw, vw, rw, ow = f(kw), f(vw), f(rw), f(ow)

    xvecs = np.zeros((P, XVECS_W), dtype=np.float32)
    xvecs[:, OFF_XK:OFF_XK + KT] = xk
    xvecs[:, OFF_XV:OFF_XV + KT] = xv
    xvecs[:, OFF_XR:OFF_XR + KT] = xr
    xvecs = xvecs.astype(NP_BF16)

    e0 = np.zeros((P, 2), dtype=np.float32)
    e0[0, 0] = 1.0

    wb = lambda a: np.ascontiguousarray(a).astype(NP_BF16)

    # WKV-side [128, 4] layout: channel = j*128 + p
    pm = lambda v: np.ascontiguousarray(v.reshape(OW_KT, P).T)
    in_maps = []
    for c in range(NCORES):
        sl = slice(c * S, (c + 1) * S)
        svecs = np.empty((P, SVECS_W), dtype=np.float32)
        svecs[:, OFF_AA:OFF_AA + 4] = pm(aa[sl])
        svecs[:, OFF_BB:OFF_BB + 4] = pm(bb[sl])
        svecs[:, OFF_PP:OFF_PP + 4] = pm(pp[sl])
        svecs[:, OFF_TF:OFF_TF + 4] = pm(tf[sl])
        svecs[:, OFF_TD:OFF_TD + 4] = pm(td[sl])
        in_maps.append({
            "xvecs": xvecs,
            "e0": e0,
            "svecs": svecs,
            "wr": wb(rw[sl, :].T).reshape(P, KT, S),
            "wk": wb(kw[sl, :].T).reshape(P, KT, S),
            "wv": wb(vw[sl, :].T).reshape(P, KT, S),
            "wo": wb(ow[:, sl].T).reshape(OW_KT, P, C),
        })
    return in_maps, x


_NC_CACHE = None


def _run(inputs, trace=False):
    global _NC_CACHE
    if _NC_CACHE is None:
        _NC_CACHE = _build()
    nc = _NC_CACHE
    in_maps, x = _prep_in_maps(**inputs)
    res = bass_utils.run_bass_kernel_spmd(
        nc, in_maps, core_ids=list(range(NCORES)), trace=trace)

    out = np.zeros(C, dtype=np.float32)
    new_a = np.empty(C, dtype=np.float32)
    new_b = np.empty(C, dtype=np.float32)
    new_p = np.empty(C, dtype=np.float32)
    for c in range(NCORES):
        r = res.results[c]
        out += r["partial"].reshape(C)
        sl = slice(c * S, (c + 1) * S)
        nst = r["nst"]
        # [p, j] -> channel j*128 + p
        new_a[sl] = nst[:, 0:4].T.reshape(S)
        new_b[sl] = nst[:, 4:8].T.reshape(S)
        new_p[sl] = nst[:, 8:12].T.reshape(S)
    return (out, x.copy(), new_a, new_b, new_p), res


def kernel(**inputs):
    outs, _ = _run(inputs, trace=False)
    return outs


# revision 7
# speedup vs baseline: 1.6076x; 1.0762x over previous
"""RWKV time-mixing (C=4096) on 8 trn2 NeuronCores.

Strategy (tensor-parallel over channels, M=8 cores, S=C/M=512):
  - Core c owns channels sl = [c*512, (c+1)*512).
  - Weights stream in bf16 (matvec is HBM-bound; bf16 halves the traffic
    to 16 MB/core and the rel-err budget of 2e-2 has ~6x margin over the
    measured bf16 error). All elementwise WKV math stays fp32, and the
    PE-reshape trick stays fp32r so kk keeps full precision into exp().
  - Phase 1: kk/vv/rr shards: kw[sl,:] @ xk etc. Weights are host-transposed
    so the contraction dim lands on SBUF partitions; the x-vector column is
    the (tiny) stationary operand, weight tiles stream as the moving operand
    at N=512 in bf16 (1 cycle/row on the PE).
  - WKV recurrence: purely elementwise on the 512-channel shard, done in a
    [128, 4] layout (channel = j*128 + p).
  - Phase 2: partial out = ow[:, sl] @ (r*wkv): 4 k-tiles x 8 n-banks of
    matmuls into a [1, 4096] PSUM row; host sums the 8 per-core partials
    (the "all-reduce" of the column-sharded matvec).
  - new_state = x exactly (token shift), assembled on host.

Stream order is MATRIX-MAJOR (all of rw, then kw, then vw, then ow) so the
dependent chain drains while later weights stream: rr is complete ~1/4 into
the stream (sigmoid runs there), kk at ~2/4 (the kk-only part of the WKV
recurrence runs there), vv at ~3/4 (the short vv tail + y = r*wkv run
there, just before the first ow chunk lands), and the phase-2 matmuls then
chase the ow chunks as they arrive. Only the last ow half-chunk's four
matmuls + PSUM copies + output DMA remain after the final weight byte.

k-index convention (phase 1): k = p*32 + n  (p = partition, n = k-tile id),
so W.T.reshape(128, 32, 512) puts k-tile n at [:, n, :] with contraction on
partitions, and x.reshape(128, 32) column n is the matching stationary vec.

The [1,512] -> [128,4] reshape of the phase-1 results runs on the PE: a
matmul whose stationary operand is a zero-padded [128,128] slice with the
data in partition 0 and whose moving operand is the unit vector e0 emits
the row slice as a [128,2] PSUM column pair (channel = j*128 + p). Phase 2
contracts k-tile tt=j over rows [j*128,(j+1)*128) of ow[:, sl].T, matching
that layout. This path is fp32r end-to-end: rounding kk to bf16 here would
put ~1% error into exp(kk) on the largest channels.
"""

import ml_dtypes
import numpy as np

import concourse.bass as bass
import concourse.mybir as mybir
import concourse.tile as tile
from concourse import bacc, bass_utils

C = 4096
NCORES = 8
S = C // NCORES          # 512 channels per core
P = 128
KT = C // P              # 32 k-tiles in phase 1
KSUB = 8                 # k-tiles per DMA chunk (1 MB bf16 chunks)
NCHUNK = KT // KSUB      # 4 chunks per phase-1 matrix
OW_KT = S // P           # 4 k-tiles in phase 2
OW_HALF = C // 2         # ow k-tile chunks split into 512 KB column halves

F32 = mybir.dt.float32
F32R = mybir.dt.float32r
BF16 = mybir.dt.bfloat16
AF = mybir.ActivationFunctionType
MM_DTYPE = BF16          # weight-stream dtype: 1 cycle/row, 2 bytes/elem
NP_BF16 = ml_dtypes.bfloat16

# xvecs layout: [128, 96] = xk[0:32] | xv[32:64] | xr[64:96] (bf16)
# e0 (fp32r) is a separate [128, 2] input: unit vector [1,0,...,0] + a zero
# column (N=2 moving operand of the PE-reshape matmuls — fp32r matmuls
# require an even moving free size)
# svecs layout: [128, 20] = aa[0:4] | bb[4:8] | pp[8:12] | tf[12:16] | td[16:20]
XVECS_W = 3 * KT
SVECS_W = 5 * 4
OFF_XK, OFF_XV, OFF_XR = 0, KT, 2 * KT
OFF_AA, OFF_BB, OFF_PP, OFF_TF, OFF_TD = 0, 4, 8, 12, 16


def _build():
    nc = bacc.Bacc("TRN2", target_bir_lowering=False, debug=False,
                   num_devices=NCORES)

    xvecs_d = nc.dram_tensor("xvecs", [P, XVECS_W], MM_DTYPE, kind="ExternalInput")
    e0_d = nc.dram_tensor("e0", [P, 2], F32R, kind="ExternalInput")
    svecs_d = nc.dram_tensor("svecs", [P, SVECS_W], F32, kind="ExternalInput")
    wr_d = nc.dram_tensor("wr", [P, KT, S], MM_DTYPE, kind="ExternalInput")
    wk_d = nc.dram_tensor("wk", [P, KT, S], MM_DTYPE, kind="ExternalInput")
    wv_d = nc.dram_tensor("wv", [P, KT, S], MM_DTYPE, kind="ExternalInput")
    wo_d = nc.dram_tensor("wo", [OW_KT, P, C], MM_DTYPE, kind="ExternalInput")

    partial_d = nc.dram_tensor("partial", [1, C], F32, kind="ExternalOutput")
    nst_d = nc.dram_tensor("nst", [P, 12], F32, kind="ExternalOutput")

    with tile.TileContext(nc) as tc:
        with (
            tc.tile_pool(name="const", bufs=1) as const,
            tc.tile_pool(name="w", bufs=12) as wpool,
            tc.tile_pool(name="ow", bufs=8) as opool,
            tc.tile_pool(name="small", bufs=1) as small,
        ):
            # z2 feeds the ramp-trigger matmul (emitted once the PSUM pool
            # opens below); its memset leads the gpsimd queue so the trigger
            # can fire as early as possible.
            z2 = small.tile([P, 2], F32)
            nc.gpsimd.memset(z2[:], 0.0)

            # ALL small DMAs ride the SWDGE (gpsimd) ring: the SP HWDGE ring
            # carries only weight traffic, keeping its DMA-completion
            # semaphore lanes clean (shared lanes across rings were observed
            # to delay ow-chunk completion sems by ~15us). Emitted BEFORE the
            # big stage_z memset so the descriptor writes are not delayed.
            xvecs = const.tile([P, XVECS_W], MM_DTYPE)
            nc.gpsimd.dma_start(xvecs[:], xvecs_d[:])
            # pinned weight tile for bridging matmuls between the ramp
            # trigger and the first chunk's arrival (keeps the PE's
            # "continuous execution" clock running toward full speed)
            filler = const.tile([P, 1, S], MM_DTYPE)
            nc.gpsimd.dma_start(filler[:], wr_d[:, 0:1, :])
            e0 = const.tile([P, 2], F32R)
            nc.gpsimd.dma_start(e0[:], e0_d[:])
            svecs = const.tile([P, SVECS_W], F32)
            nc.gpsimd.dma_start(svecs[:], svecs_d[:])

            # preload the ACT exp LUT off the critical path (the only ACT
            # table the kernel uses: sigmoid is computed via exp+reciprocal)
            warm = small.tile([1, 4], F32)
            nc.gpsimd.memset(warm[:], 0.0)
            warm2 = small.tile([1, 4], F32)
            nc.scalar.activation(warm2[:], warm[:], AF.Exp)

            # stage for the PE reshape: partition 0 carries the phase-1
            # results, rows 1-127 must be finite (they multiply e0's zeros).
            # f32r memset is invalid ISA, so zero an f32 twin and cast-copy.
            stage_z = small.tile([P, 3 * S], F32)
            nc.gpsimd.memset(stage_z[:], 0.0)
            stage = small.tile([P, 3 * S], F32R)
            nc.vector.tensor_copy(stage[:], stage_z[:])

            aa = svecs[:, OFF_AA:OFF_AA + 4]
            bb = svecs[:, OFF_BB:OFF_BB + 4]
            pp = svecs[:, OFF_PP:OFF_PP + 4]
            tf = svecs[:, OFF_TF:OFF_TF + 4]
            td = svecs[:, OFF_TD:OFF_TD + 4]

            def t4(name):
                return small.tile([P, 4], F32, name=name)

            nst = small.tile([P, 12], F32)
            na, nb_t, p2 = nst[:, 0:4], nst[:, 4:8], nst[:, 8:12]
            rkv = small.tile([P, 12], F32)
            rr128 = rkv[:, 0:4]
            kk = rkv[:, 4:8]
            vv = rkv[:, 8:12]

            # ---- phase 1 + overlapped recurrence ---------------------------
            wdrams = [wr_d, wk_d, wv_d]
            xoffs = [OFF_XR, OFF_XK, OFF_XV]
            # WKV temporaries shared between the kk-stage and the vv-stage
            r128 = small.tile([P, 4], F32, name="r128")
            e2a = t4("e2a")
            e2b = t4("e2b")
            acc_a = t4("acc_a")
            binv = t4("binv")
            y = t4("y")

            with tc.tile_pool(name="ps1", bufs=1, space="PSUM") as ps1:
                psums = [ps1.tile([1, S], F32, name=f"ps_{i}") for i in range(3)]
                fl_ps = ps1.tile([1, S], F32, name="fl_ps")
                rs_ps = ps1.tile([P, 24], F32, name="rs_ps")

                # ramp trigger: the PE stalls ~5.4us in a p-state transition
                # after its FIRST matmul, then runs at half speed for ~3us.
                # Fire a tiny f32 matmul as early as possible (the z2 memset
                # is its only dependency) so the stall completes right as the
                # first weight chunk lands.
                zps = ps1.tile([2, 2], F32, name="zps")
                nc.tensor.matmul(zps[:], lhsT=z2[:], rhs=z2[:], start=True, stop=True)

                def fill_mm(n):
                    for i in range(n):
                        nc.tensor.matmul(
                            fl_ps[:],
                            lhsT=xvecs[:, 0:1],
                            rhs=filler[:, 0, :],
                            start=True,
                            stop=True,
                        )

                # bridge the PE from the ramp trigger to the first chunk
                fill_mm(2)
                for wi in range(3):
                    for chunk in range(NCHUNK):
                        wt = wpool.tile([P, KSUB, S], MM_DTYPE, tag="wchunk")
                        nc.sync.dma_start(
                            wt[:], wdrams[wi][:, chunk * KSUB:(chunk + 1) * KSUB, :])
                        for tl in range(KSUB):
                            kt = chunk * KSUB + tl
                            nc.tensor.matmul(
                                psums[wi][:],
                                lhsT=xvecs[:, xoffs[wi] + kt:xoffs[wi] + kt + 1],
                                rhs=wt[:, tl, :],
                                start=(kt == 0),
                                stop=(kt == KT - 1),
                            )

                    # matrix wi fully reduced: transpose its [1,512] row into
                    # the [128,4] WKV layout while the next matrix streams
                    if wi == 0:
                        nc.scalar.copy(stage[0:1, 0:S], psums[0][:])
                    else:
                        nc.vector.tensor_copy(
                            stage[0:1, wi * S:(wi + 1) * S], psums[wi][:])
                    for j in range(OW_KT):
                        c2 = 2 * (wi * 4 + j)
                        nc.tensor.matmul(
                            rs_ps[:, c2:c2 + 2],
                            lhsT=stage[:, wi * S + j * P:wi * S + (j + 1) * P],
                            rhs=e0[:],
                            start=True,
                            stop=True,
                        )
                    nc.vector.tensor_copy(
                        rkv[:, wi * 4:wi * 4 + 4],
                        rs_ps[:, 2 * wi * 4:2 * wi * 4 + 8:2])

                    if wi == 0:
                        # r = sigmoid(rr) = 1 / (1 + exp(-rr)) — exp only
                        er = t4("er")
                        nc.scalar.activation(er[:], rr128, AF.Exp, scale=-1.0)
                        rp1 = t4("rp1")
                        nc.vector.tensor_scalar_add(rp1[:], er[:], 1.0)
                        nc.vector.reciprocal(r128[:], rp1[:])
                    elif wi == 1:
                        # everything in the WKV recurrence that needs only kk
                        ww1 = t4("ww1")
                        nc.vector.tensor_add(ww1, tf, kk[:])
                        p1 = t4("p1")
                        nc.vector.tensor_max(p1, pp, ww1)
                        d1 = t4("d1")
                        nc.vector.tensor_sub(d1, pp, p1)
                        e1a = t4("e1a")
                        nc.scalar.activation(e1a, d1, AF.Exp)
                        d2 = t4("d2")
                        nc.vector.tensor_sub(d2, ww1, p1)
                        nc.scalar.activation(e2a[:], d2, AF.Exp)
                        nc.vector.tensor_mul(acc_a[:], e1a, aa)   # e1*aa
                        acc_b = t4("acc_b")
                        nc.vector.tensor_mul(acc_b, e1a, bb)
                        nc.vector.tensor_add(acc_b, acc_b, e2a[:])
                        nc.vector.reciprocal(binv[:], acc_b)
                        # state update, kk-only part
                        ww2 = t4("ww2")
                        nc.vector.tensor_add(ww2, pp, td)
                        nc.vector.tensor_max(p2, ww2, kk[:])
                        d3 = t4("d3")
                        nc.vector.tensor_sub(d3, ww2, p2)
                        e1b = t4("e1b")
                        nc.scalar.activation(e1b, d3, AF.Exp)
                        d4 = t4("d4")
                        nc.vector.tensor_sub(d4, kk[:], p2)
                        nc.scalar.activation(e2b[:], d4, AF.Exp)
                        nc.vector.tensor_mul(na, e1b, aa)         # e1*aa
                        nc.vector.tensor_mul(nb_t, e1b, bb)
                        nc.vector.tensor_add(nb_t, nb_t, e2b[:])
                    else:
                        # short vv tail: a = e1*aa + e2*vv, y = r * a/b
                        tmp_a = t4("tmp_a")
                        nc.vector.tensor_mul(tmp_a, e2a[:], vv[:])
                        nc.vector.tensor_add(acc_a[:], acc_a[:], tmp_a)
                        nc.vector.tensor_mul(y[:], acc_a[:], binv[:])  # wkv
                        nc.vector.tensor_mul(y[:], r128[:], y[:])      # r*wkv
                        tmp_b = t4("tmp_b")
                        nc.vector.tensor_mul(tmp_b, e2b[:], vv[:])
                        nc.vector.tensor_add(na, na, tmp_b)

                # ow weight DMAs: the in-order HWDGE ring plays them right
                # after the phase-1 weight DMAs, by which time y is ready and
                # the phase-2 matmuls chase the arriving chunks.
                otiles = {}
                for tt in range(OW_KT):
                    for half in range(2):
                        ot = opool.tile([P, OW_HALF], MM_DTYPE, tag="owchunk")
                        nc.sync.dma_start(
                            ot[:], wo_d[tt][:, half * OW_HALF:(half + 1) * OW_HALF])
                        otiles[(tt, half)] = ot

            nc.gpsimd.dma_start(nst_d[:], nst[:])

            # round y to bf16 for the ow matmuls (operand dtypes must match)
            y_r = small.tile([P, 4], MM_DTYPE)
            nc.vector.tensor_copy(y_r[:], y[:])

            # ---- phase 2: partial = ow[:, sl] @ y ---------------------------
            with tc.tile_pool(name="ps2", bufs=1, space="PSUM") as ps2:
                ow_ps = ps2.tile([1, C], F32)
                out_sb = small.tile([1, C], F32)
                for tt in range(OW_KT):
                    for nb in range(C // 512):
                        half, col = nb // 4, nb % 4
                        nc.tensor.matmul(
                            ow_ps[:, nb * 512:(nb + 1) * 512],
                            lhsT=y_r[:, tt:tt + 1],
                            rhs=otiles[(tt, half)][:, col * 512:(col + 1) * 512],
                            start=(tt == 0),
                            stop=(tt == OW_KT - 1),
                        )
                        if tt == OW_KT - 1:
                            # bank nb is complete; copy out while later banks
                            # are still accumulating
                            sl_ = slice(nb * 512, (nb + 1) * 512)
                            if nb % 2 == 0:
                                nc.vector.tensor_copy(out_sb[:, sl_], ow_ps[:, sl_])
                            else:
                                nc.scalar.copy(out_sb[:, sl_], ow_ps[:, sl_])
                            if nb == 3:
                                # first half of the output leaves while banks
                                # 4-7 are still accumulating
                                nc.gpsimd.dma_start(
                                    partial_d[:, 0:C // 2], out_sb[:, 0:C // 2])

            nc.gpsimd.dma_start(partial_d[:, C // 2:], out_sb[:, C // 2:])

    nc.compile()
    return nc


def _prep_in_maps(x, state, state_a, state_b, state_p,
                  time_mix_k, time_mix_v, time_mix_r,
                  time_first, time_decay, kw, vw, rw, ow):
    f = lambda a: np.ascontiguousarray(np.asarray(a), dtype=np.float32)
    x, state = f(x), f(state)
    tmk, tmv, tmr = f(time_mix_k), f(time_mix_v), f(time_mix_r)
    xk = (x * tmk + state * (1.0 - tmk)).reshape(P, KT)
    xv = (x * tmv + state * (1.0 - tmv)).reshape(P, KT)
    xr = (x * tmr + state * (1.0 - tmr)).reshape(P, KT)
    aa, bb, pp = f(state_a), f(state_b), f(state_p)
    tf, td = f(time_first), f(time_decay)
    kw, vw, rw, ow = f(kw), f(vw), f(rw), f(ow)

    xvecs = np.zeros((P, XVECS_W), dtype=np.float32)
    xvecs[:, OFF_XK:OFF_XK + KT] = xk
    xvecs[:, OFF_XV:OFF_XV + KT] = xv
    xvecs[:, OFF_XR:OFF_XR + KT] = xr
    xvecs = xvecs.astype(NP_BF16)

    e0 = np.zeros((P, 2), dtype=np.float32)
    e0[0, 0] = 1.0

    wb = lambda a: np.ascontiguousarray(a).astype(NP_BF16)

    # WKV-side [128, 4] layout: channel = j*128 + p
    pm = lambda v: np.ascontiguousarray(v.reshape(OW_KT, P).T)
    in_maps = []
    for c in range(NCORES):
        sl = slice(c * S, (c + 1) * S)
        svecs = np.empty((P, SVECS_W), dtype=np.float32)
        svecs[:, OFF_AA:OFF_AA + 4] = pm(aa[sl])
        svecs[:, OFF_BB:OFF_BB + 4] = pm(bb[sl])
        svecs[:, OFF_PP:OFF_PP + 4] = pm(pp[sl])
        svecs[:, OFF_TF:OFF_TF + 4] = pm(tf[sl])
        svecs[:, OFF_TD:OFF_TD + 4] = pm(td[sl])
        in_maps.append({
            "xvecs": xvecs,
            "e0": e0,
            "svecs": svecs,
            "wr": wb(rw[sl, :].T).reshape(P, KT, S),
            "wk": wb(kw[sl, :].T).reshape(P, KT, S),
            "wv": wb(vw[sl, :].T).reshape(P, KT, S),
            "wo": wb(ow[:, sl].T).reshape(OW_KT, P, C),
        })
    return in_maps, x


_NC_CACHE = None


def _run(inputs, trace=False):
    global _NC_CACHE
    if _NC_CACHE is None:
        _NC_CACHE = _build()
    nc = _NC_CACHE
    in_maps, x = _prep_in_maps(**inputs)
    res = bass_utils.run_bass_kernel_spmd(
        nc, in_maps, core_ids=list(range(NCORES)), trace=trace)

    out = np.zeros(C, dtype=np.float32)
    new_a = np.empty(C, dtype=np.float32)
    new_b = np.empty(C, dtype=np.float32)
    new_p = np.empty(C, dtype=np.float32)
    for c in range(NCORES):
        r = res.results[c]
        out += r["partial"].reshape(C)
        sl = slice(c * S, (c + 1) * S)
        nst = r["nst"]
        # [p, j] -> channel j*128 + p
        new_a[sl] = nst[:, 0:4].T.reshape(S)
        new_b[sl] = nst[:, 4:8].T.reshape(S)
        new_p[sl] = nst[:, 8:12].T.reshape(S)
    return (out, x.copy(), new_a, new_b, new_p), res


def kernel(**inputs):
    outs, _ = _run(inputs, trace=False)
    return outs
